# revision 22
# baseline (speedup 1.0000x reference)
"""Trainium2 Bass kernel for nn_ByteToLatentAttention.

Sharding: 8 cores = 2 (batch) x 4 (head-groups of 4 heads).  Each core
computes a partial output  attn_part @ wout_rows + merged_raw_rows @ wbyp_rows
for its batch; the host sums the 4 partials per batch and adds wout_b.

Precision: bypass path f32r (dominates output magnitude); attention path
bf16 with fp32 PSUM.  fp8/DoubleRow was tried and reverted: it trips the
HAM activity-1 throttle (50% engine duty cycle), which exactly cancels the
2x instruction saving and slows every other engine.

Structure vs the original baseline:
  * attn@V runs in [q, d] orientation (q on PSUM partitions) with a ones
    column appended to V, so the softmax denominator falls out of the same
    matmuls (no separate denominator matmuls / esum adds) and the divide
    is a per-partition scalar_tensor_tensor that also folds in the V bias.
  * attention output is transposed back via 16 small PE transposes.
  * the scalar engine runs (almost) only the softmax exp; psum->sbuf
    evacuations and rope elementwise work are spread over DVE and GpSimd.
  * scores->exp->attnV is pipelined per 128-k-chunk with a 3-deep PSUM
    rotation; out-proj work for q-half 0 is interleaved into the attention
    of q-half 1 to keep the PE from idling (p-state) during exp waits.

Self-contained: hardcodes all shapes; uses only numpy + concourse.
"""

from contextlib import ExitStack

import numpy as np

import concourse.bass as bass
import concourse.tile as tile
from concourse import bacc
from concourse import mybir
from concourse.bass_utils import run_bass_kernel_spmd
from concourse.masks import make_identity

# ---- problem constants ----
B, S, D = 2, 4096, 512
BPL, H, DQK = 4, 16, 64
DLAT = 1024
LQ = S // BPL  # 1024
EPS = 1.1920929e-07
ROPE_BASE = 10000.0
NCORES = 8
NH = (H // 4) * DQK  # 256 features per core (4 heads)
P = 128

F32 = mybir.dt.float32
BF16 = mybir.dt.bfloat16
MM_F32 = mybir.dt.float32r

AF = mybir.ActivationFunctionType
ALU = mybir.AluOpType


def _kernel_body(ctx: ExitStack, tc, io):
    nc = tc.nc

    const = ctx.enter_context(tc.tile_pool(name="const", bufs=1))
    work = ctx.enter_context(tc.tile_pool(name="work", bufs=2))
    evp = ctx.enter_context(tc.tile_pool(name="evp", bufs=3))
    stage = ctx.enter_context(tc.tile_pool(name="stage", bufs=1))
    psS = ctx.enter_context(tc.tile_pool(name="psS", bufs=3, space="PSUM"))
    psAcc = ctx.enter_context(tc.tile_pool(name="psAcc", bufs=1, space="PSUM"))

    ident_h = const.tile([P, P], BF16)
    make_identity(nc, ident_h)
    ones128 = const.tile([P, P], BF16)
    nc.vector.memset(ones128, 1.0)
    eps_sb = const.tile([P, 1], F32)
    nc.vector.memset(eps_sb, EPS)
    rot_sb = const.tile([P, P], BF16)
    nc.sync.dma_start(out=rot_sb, in_=io["rotm"])

    bq_sb = const.tile([P, 2], F32)
    nc.sync.dma_start(out=bq_sb, in_=io["bq"])
    bk_sb = const.tile([P, 2], F32)
    nc.sync.dma_start(out=bk_sb, in_=io["bk"])
    bvrow = const.tile([P, NH], BF16)
    nc.sync.dma_start(out=bvrow, in_=io["bvrow"])

    # persistent big tensors
    bypT = const.tile([P, 4, LQ], MM_F32)  # [d_p, dc, l]  raw bypass rows^T
    QTr = const.tile([P, 2, LQ], BF16)  # roped Q^T (pair hp -> heads 2hp,2hp+1)
    KTr = const.tile([P, 2, S], BF16)  # roped K^T
    Vn = const.tile([P, 32, 4 * 65], BF16)  # [s_p, sc, 65*h(+den col)]
    acT = const.tile([P, 2, LQ], BF16)  # [d_p, half, q]  attn^T

    VnH = Vn.rearrange("p c (h x) -> p c h x", x=65)
    # denominator column of ones interleaved at 65h+64
    nc.vector.memset(VnH[:, :, :, 64], 1.0)

    # ---------- phase 0: PE warm-up (p-state ramp) ----------
    for wi in range(32):
        warm_ps = psS.tile([P, 1024], F32, tag="sc")
        nc.tensor.matmul(
            warm_ps[:, 0:128], lhsT=ident_h, rhs=ident_h, start=True, stop=True
        )

    # ---------- phase 1: RMS norm in transposed domain ----------
    # xT is normalized IN PLACE and serves as normXT afterwards
    xT = stage.tile([P, 4, S], BF16, tag="xT")
    nc.sync.dma_start(out=xT, in_=io["x_b"])
    nc.sync.dma_start(out=bypT, in_=io["x_byp"])
    normXT = xT

    for sc8 in range(8):
        ssl = slice(sc8 * 512, (sc8 + 1) * 512)
        sq = work.tile([P, 4, 512], BF16, tag="sq")
        for dc in range(4):
            eng = nc.gpsimd if dc % 2 == 0 else nc.vector
            eng.tensor_mul(out=sq[:, dc, :], in0=xT[:, dc, ssl], in1=xT[:, dc, ssl])
        pss = psS.tile([P, 1024], F32, tag="sc")
        for dc in range(4):
            nc.tensor.matmul(
                pss[:, 0:512],
                lhsT=ones128,
                rhs=sq[:, dc, :],
                start=(dc == 0),
                stop=(dc == 3),
            )
        rmsb = work.tile([P, 512], F32, tag="rmsb")
        nc.scalar.activation(
            out=rmsb, in_=pss[:, 0:512], func=AF.Sqrt, bias=eps_sb, scale=1.0 / D
        )
        rinvf = work.tile([P, 512], F32, tag="rinvf")
        nc.vector.reciprocal_approx_fast(out=rinvf, in_=rmsb)
        for dc in range(4):
            eng = nc.gpsimd if dc % 2 == 0 else nc.vector
            # in-place normalize: xT <- xT * rinv
            eng.tensor_mul(out=xT[:, dc, ssl], in0=xT[:, dc, ssl], in1=rinvf)

    # ---------- phase 2: K projection + rope ----------
    wk_sb = stage.tile([P, 4, NH], BF16, tag="wk")
    nc.sync.dma_start(out=wk_sb, in_=io["wk"])
    csk = stage.tile([P, 2, S], BF16, tag="csk")
    nc.sync.dma_start(out=csk[:, 0, :], in_=io["cosk"])
    nc.sync.dma_start(out=csk[:, 1, :], in_=io["sink"])

    def rope_tile(pk, dstT, mk, ssl, cos, sin, bias):
        # kb = pk + bias; rot = rotm^T @ kb; dst = kb*cos + rot*sin
        kb = work.tile([P, 512], BF16, tag="kb")
        nc.vector.tensor_scalar_add(out=kb, in0=pk, scalar1=bias[:, mk : mk + 1])
        pr = psS.tile([P, 1024], F32, tag="sc")
        nc.tensor.matmul(pr[:, 0:512], lhsT=rot_sb, rhs=kb, start=True, stop=True)
        prh = work.tile([P, 512], BF16, tag="prh")
        nc.vector.tensor_copy(out=prh, in_=pr[:, 0:512])
        t1 = work.tile([P, 512], BF16, tag="t1")
        nc.gpsimd.tensor_mul(out=t1, in0=kb, in1=cos)
        t2 = work.tile([P, 512], BF16, tag="t2")
        nc.gpsimd.tensor_mul(out=t2, in0=prh, in1=sin)
        nc.gpsimd.tensor_add(out=dstT[:, mk, ssl], in0=t1, in1=t2)

    for sf in range(8):
        ssl = slice(sf * 512, (sf + 1) * 512)
        for mk in range(2):
            pk = psS.tile([P, 1024], F32, tag="sc")
            for dc in range(4):
                nc.tensor.matmul(
                    pk[:, 0:512],
                    lhsT=wk_sb[:, dc, mk * P : (mk + 1) * P],
                    rhs=normXT[:, dc, ssl],
                    start=(dc == 0),
                    stop=(dc == 3),
                )
            rope_tile(pk[:, 0:512], KTr, mk, ssl, csk[:, 0, ssl], csk[:, 1, ssl], bk_sb)

    # ---------- phase 3: Q projection + rope ----------
    wq_sb = stage.tile([P, 16, NH], BF16, tag="wq")
    nc.sync.dma_start(out=wq_sb, in_=io["wq"])
    csq = stage.tile([P, 2, LQ], BF16, tag="csq")
    nc.sync.dma_start(out=csq[:, 0, :], in_=io["cosq"])
    nc.sync.dma_start(out=csq[:, 1, :], in_=io["sinq"])

    normQ = normXT.rearrange("p c (l j) -> p c l j", j=4)

    for qf in range(2):
        qsl = slice(qf * 512, (qf + 1) * 512)
        for mq in range(2):
            pq = psS.tile([P, 1024], F32, tag="sc")
            for kc in range(16):
                jj, dc = kc // 4, kc % 4
                nc.tensor.matmul(
                    pq[:, 0:512],
                    lhsT=wq_sb[:, kc, mq * P : (mq + 1) * P],
                    rhs=normQ[:, dc, qsl, jj],
                    start=(kc == 0),
                    stop=(kc == 15),
                )
            rope_tile(pq[:, 0:512], QTr, mq, qsl, csq[:, 0, qsl], csq[:, 1, qsl], bq_sb)

    # ---------- phase 4: V projection ----------
    wv_sb = stage.tile([P, 4, NH], BF16, tag="wv")
    nc.sync.dma_start(out=wv_sb, in_=io["wv"])
    for sc in range(S // P):
        pv = psS.tile([P, 1024], F32, tag="sc")
        for dc in range(4):
            nc.tensor.matmul(
                pv[:, 0:NH],
                lhsT=normXT[:, dc, sc * P : (sc + 1) * P],
                rhs=wv_sb[:, dc, :],
                start=(dc == 0),
                stop=(dc == 3),
            )
        nc.vector.tensor_copy(
            out=VnH[:, sc, :, 0:64],
            in_=pv[:, 0:NH].rearrange("p (h x) -> p h x", x=64),
        )

    # ---------- phase 5: attention ----------
    wo_sb = stage.tile([P, 2, DLAT], BF16, tag="wo")
    nc.sync.dma_start(out=wo_sb, in_=io["wo"])
    wb_sb = stage.tile([P, 4, DLAT], MM_F32, tag="wb")
    nc.sync.dma_start(out=wb_sb, in_=io["wb"])

    pe_backlog = []  # deferred PE+evac work to interleave into attention

    def attention_block(qc, hp):
        # q block [qc*512, (qc+1)*512), heads (2hp, 2hp+1)
        qsl = slice(qc * 512, (qc + 1) * 512)
        pac = psAcc.tile([P, 2, 512], F32, tag="pac")

        def scores(sc):
            # scores + exp for one 128-k chunk -> e [128k, 2head*512q] bf16
            ksl = slice(sc * P, (sc + 1) * P)
            ps = psS.tile([P, 1024], F32, tag="sc")
            nc.tensor.matmul(
                ps[:, 0:512],
                lhsT=KTr[0:64, hp, ksl],
                rhs=QTr[0:64, hp, qsl],
                start=True,
                stop=True,
                skip_group_check=True,
            )
            nc.tensor.matmul(
                ps[:, 512:1024],
                lhsT=KTr[64:128, hp, ksl],
                rhs=QTr[64:128, hp, qsl],
                start=True,
                stop=True,
                skip_group_check=True,
            )
            e = evp.tile([P, 1024], BF16, tag="e")
            nc.scalar.activation(out=e, in_=ps, func=AF.Exp, scale=0.125)
            return e

        def attnv(sc, e):
            st, sp = (sc == 0), (sc == 31)
            for h in range(2):
                for qt in range(4):
                    nc.tensor.matmul(
                        pac[:, h, qt * 128 : qt * 128 + 65],
                        lhsT=e[:, h * 512 + qt * 128 : h * 512 + (qt + 1) * 128],
                        rhs=VnH[:, sc, 2 * hp + h, :],
                        start=st,
                        stop=sp,
                        skip_group_check=True,
                    )

        e_cur = scores(0)
        for sc in range(32):
            e_next = scores(sc + 1) if sc < 31 else None
            attnv(sc, e_cur)
            e_cur = e_next
            if sc % 2 == 1 and pe_backlog:
                pe_backlog.pop(0)()
        return pac

    def divide_block(qc, hp, pac, attnN):
        # attnN[:, qt, 64*(2hp+h)+j] = pac_num * (1/den) + bv
        for h in range(2):
            hh = 2 * hp + h
            for qt in range(4):
                rc = work.tile([P, 1], F32, tag="rc")
                nc.vector.reciprocal_approx_fast(
                    out=rc, in_=pac[:, h, qt * 128 + 64 : qt * 128 + 65]
                )
                nc.vector.scalar_tensor_tensor(
                    out=attnN[:, qt, 64 * hh : 64 * hh + 64],
                    in0=pac[:, h, qt * 128 : qt * 128 + 64],
                    scalar=rc,
                    in1=bvrow[:, 64 * hh : 64 * hh + 64],
                    op0=ALU.mult,
                    op1=ALU.add,
                )

    def transpose_evac(qc, attnN):
        # attnN [128q, qt, 256d] -> acT [128d, half, q] via PE transpose
        for qt in range(4):
            for dh in range(2):
                def go(qt=qt, dh=dh):
                    psT = psS.tile([P, 1024], F32, tag="sc")
                    nc.tensor.matmul(
                        psT[:, 0:128],
                        lhsT=attnN[:, qt, dh * 128 : (dh + 1) * 128],
                        rhs=ident_h,
                        start=True,
                        stop=True,
                    )
                    qoff = qc * 512 + qt * 128
                    nc.vector.tensor_copy(
                        out=acT[:, dh, qoff : qoff + 128], in_=psT[:, 0:128]
                    )
                pe_backlog.append(go)

    def outproj(qc):
        for qt in range(4):
            qoff = qc * 512 + qt * 128

            def go(qoff=qoff):
                osb = evp.tile([P, DLAT], F32, tag="osb")
                for oc in range(2):
                    osl = slice(oc * 512, (oc + 1) * 512)
                    po = psS.tile([P, 1024], F32, tag="sc")
                    for pl in range(2):
                        nc.tensor.matmul(
                            po[:, 0:512],
                            lhsT=acT[:, pl, qoff : qoff + 128],
                            rhs=wo_sb[:, pl, osl],
                            start=(pl == 0),
                            stop=False,
                        )
                    for dc in range(4):
                        nc.tensor.matmul(
                            po[:, 0:512],
                            lhsT=bypT[:, dc, qoff : qoff + 128],
                            rhs=wb_sb[:, dc, osl],
                            start=False,
                            stop=(dc == 3),
                        )
                    nc.vector.tensor_copy(out=osb[:, osl], in_=po[:, 0:512])
                nc.sync.dma_start(
                    out=io["out_partial"][qoff : qoff + 128, :], in_=osb
                )

            pe_backlog.append(go)

    attnN_tiles = {}
    for qc in range(2):
        for hp in range(2):
            pac = attention_block(qc, hp)
            if hp == 0:
                attnN = work.tile([P, 4, NH], BF16, tag=f"attnN{qc}", name=f"attnN{qc}")
                attnN_tiles[qc] = attnN
            divide_block(qc, hp, pac, attnN_tiles[qc])
        transpose_evac(qc, attnN_tiles[qc])
        outproj(qc)
    while pe_backlog:
        pe_backlog.pop(0)()


def build_program():
    nc = bacc.Bacc("TRN2", target_bir_lowering=False, debug=False)
    io = {}

    def inp(name, shape, dtype=F32):
        io[name] = nc.dram_tensor(name, list(shape), dtype, kind="ExternalInput").ap()

    inp("x_b", [P, 4, S], BF16)
    inp("x_byp", [P, 4, LQ], MM_F32)
    inp("wq", [P, 16, NH], BF16)
    inp("wk", [P, 4, NH], BF16)
    inp("wv", [P, 4, NH], BF16)
    inp("bq", [P, 2])
    inp("bk", [P, 2])
    inp("bvrow", [P, NH], BF16)
    inp("wo", [P, 2, DLAT], BF16)
    inp("wb", [P, 4, DLAT], MM_F32)
    inp("cosq", [P, LQ], BF16)
    inp("sinq", [P, LQ], BF16)
    inp("cosk", [P, S], BF16)
    inp("sink", [P, S], BF16)
    inp("rotm", [P, P], BF16)
    io["out_partial"] = nc.dram_tensor(
        "out_partial", [LQ, DLAT], F32, kind="ExternalOutput"
    ).ap()

    with tile.TileContext(nc) as tc:
        with ExitStack() as ctx:
            _kernel_body(ctx, tc, io)
    nc.compile()
    return nc


def _chunked_rows(w, dtype):
    """[C*128, N] -> [128, C, N] (partition-major chunks for direct DMA)."""
    c = w.shape[0] // P
    return np.ascontiguousarray(w.reshape(c, P, -1).transpose(1, 0, 2).astype(dtype))


def _rope_tables(pos):
    half = DQK // 2
    invfreq = ROPE_BASE ** (-np.arange(half, dtype=np.float64) / half)
    ang = pos[:, None].astype(np.float64) * invfreq[None, :]
    cos = np.cos(ang)
    sin = np.sin(ang)
    cos64 = np.concatenate([cos, cos], axis=1).T  # [64, L]
    sin64 = np.concatenate([-sin, sin], axis=1).T
    cosT = np.concatenate([cos64, cos64], axis=0)
    sinT = np.concatenate([sin64, sin64], axis=0)
    return cosT, sinT


def _tf32(a):
    u = np.ascontiguousarray(np.asarray(a, dtype=np.float32)).view(np.uint32)
    lsb = (u >> np.uint32(13)) & np.uint32(1)
    u = (u + np.uint32(0x0FFF) + lsb) & np.uint32(0xFFFFE000)
    return u.view(np.float32)


def _bf16(a):
    import ml_dtypes

    return np.ascontiguousarray(np.asarray(a).astype(ml_dtypes.bfloat16))


def make_in_map(core, inputs):
    b, hg = core // 4, core % 4
    x = np.asarray(inputs["x"], dtype=np.float32)
    nw = np.asarray(inputs["norm_w"], dtype=np.float32)
    wq_w = np.asarray(inputs["wq_w"], dtype=np.float32)
    wq_b = np.asarray(inputs["wq_b"], dtype=np.float32)
    wkv_w = np.asarray(inputs["wkv_w"], dtype=np.float32)
    wkv_b = np.asarray(inputs["wkv_b"], dtype=np.float32)
    wout_w = np.asarray(inputs["wout_w"], dtype=np.float32)
    wbyp_w = np.asarray(inputs["wbyp_w"], dtype=np.float32)

    import ml_dtypes

    BF = ml_dtypes.bfloat16
    nsl = slice(hg * NH, (hg + 1) * NH)
    vsl = slice(H * DQK + hg * NH, H * DQK + (hg + 1) * NH)
    wq_c = wq_w * np.tile(nw, BPL)[:, None]
    wkv_c = wkv_w * nw[:, None]

    cosq, sinq = _rope_tables(np.arange(LQ) * float(BPL))
    cosk, sink = _rope_tables(np.arange(S).astype(np.float64))

    rotm = np.zeros((P, P), dtype=np.float32)
    for m in range(P):
        blk, d = (m // 64) * 64, m % 64
        rotm[blk + (d + 32) % 64, m] = 1.0

    bvr = np.tile(wkv_b[vsl][None, :], (P, 1))

    return {
        "x_b": _bf16(x[b].T.reshape(4, P, S).transpose(1, 0, 2)),
        "x_byp": _tf32(
            np.ascontiguousarray(x[b, hg::BPL, :].T.reshape(4, P, LQ).transpose(1, 0, 2))
        ),
        "wq": _chunked_rows(wq_c[:, nsl], BF),
        "wk": _chunked_rows(wkv_c[:, nsl], BF),
        "wv": _chunked_rows(wkv_c[:, vsl], BF),
        "bq": np.ascontiguousarray(wq_b[nsl].reshape(2, P).T),
        "bk": np.ascontiguousarray(wkv_b[nsl].reshape(2, P).T),
        "bvrow": _bf16(bvr),
        "wo": _chunked_rows(wout_w[nsl, :], BF),
        "wb": _tf32(_chunked_rows(wbyp_w[hg * D : (hg + 1) * D, :], np.float32)),
        "cosq": _bf16(cosq),
        "sinq": _bf16(sinq),
        "cosk": _bf16(cosk),
        "sink": _bf16(sink),
        "rotm": _bf16(rotm),
    }


_nc_cache = None


def _get_program():
    global _nc_cache
    if _nc_cache is None:
        _nc_cache = build_program()
    return _nc_cache


def run_device(inputs, trace=False):
    nc = _get_program()
    in_maps = [make_in_map(c, inputs) for c in range(NCORES)]
    res = run_bass_kernel_spmd(nc, in_maps, core_ids=list(range(NCORES)), trace=trace)
    return res


def assemble(parts, inputs):
    wout_b = np.asarray(inputs["wout_b"], dtype=np.float32)
    out = np.zeros((B, LQ, DLAT), dtype=np.float64)
    for c in range(NCORES):
        out[c // 4] += np.asarray(parts[c], dtype=np.float64)
    out += wout_b[None, None, :].astype(np.float64)
    return out.astype(np.float32)


def kernel(**inputs):
    res = run_device(inputs)
    parts = [r["out_partial"] for r in res.results]
    return assemble(parts, inputs)
